# revision 1
# baseline (speedup 1.0000x reference)
"""Single-head causal self-attention on 8 Trainium2 NeuronCores.

Reference computation (per batch b):
    k = x @ Wk.T ; q = x @ Wq.T ; v = x @ Wv.T
    wei = softmax(mask(q @ k.T / sqrt(H)))
    out = wei @ v

Strategy:
  - Data parallel: shard B=256 across 8 cores (32 batches each), replicate
    weights. No cross-core communication.
  - Algebraic fusion: q @ k.T = x (Wq.T Wk) x.T.  G = Wq.T @ Wk * scale is
    precomputed once on-chip (9 matmuls), which halves the per-batch q/k
    projection work (2 * T*C*H  ->  T*C*C with C==H plus amortized G).
  - Scores are computed directly in transposed layout ST[s, t] so that the
    attention output matmul can consume exp(ST) as the stationary operand
    without any per-batch transpose of the weights matrix.
  - Softmax denominator: V is augmented with a ones column, so the output
    matmul also produces r[t] = sum_s exp(ST[s,t]); the final normalization
    is a per-partition reciprocal multiply.  No max-subtraction is needed:
    logits are ~N(0,1) scaled, |logit| < ~10, exp() is safe in fp32.
  - All matmuls run as float32r (TF32-like) at 1 cycle/row (4x faster than
    fp32) with fp32 PSUM accumulation.
"""

import numpy as np

import concourse.bass as bass
import concourse.mybir as mybir
from concourse import bacc
import concourse.tile as tile
from concourse.bass_utils import run_bass_kernel_spmd
from concourse.masks import make_identity

B, T, C, H = 256, 256, 384, 384
NCORES = 8
NB = B // NCORES  # batches per core
P = 128
CC = C // P  # 3 chunks of the embedding dim
TC = T // P  # 2 chunks of the sequence dim
SCALE = float(H) ** -0.5
F32 = mybir.dt.float32
F32R = mybir.dt.float32r

USE_F32R = True
MM_DT = F32R if USE_F32R else F32


def build_bass(nb: int = NB):
    nc = bacc.Bacc(
        "TRN2",
        target_bir_lowering=False,
        debug=False,
        enable_asserts=False,
        num_devices=NCORES,
    )
    x_d = nc.dram_tensor("x", [nb, T, C], F32, kind="ExternalInput").ap()
    wk_d = nc.dram_tensor("Wk", [H, C], F32, kind="ExternalInput").ap()
    wq_d = nc.dram_tensor("Wq", [H, C], F32, kind="ExternalInput").ap()
    wv_d = nc.dram_tensor("Wv", [H, C], F32, kind="ExternalInput").ap()
    out_d = nc.dram_tensor("out", [nb, T, H], F32, kind="ExternalOutput").ap()

    with tile.TileContext(nc) as tc:
        with (
            tc.tile_pool(name="const", bufs=1) as cpool,
            tc.tile_pool(name="sb", bufs=3) as sb,
            tc.tile_pool(name="ob", bufs=4) as obp,
            tc.tile_pool(name="pt", bufs=2, space="PSUM") as ptp,
            tc.tile_pool(name="pm", bufs=5, space="PSUM") as pmp,
        ):
            ident = cpool.tile([P, P], F32, name="ident")
            make_identity(nc, ident)

            # mask[sc][p, t] = 1.0 where (s = sc*128 + p) <= t else 0.0
            masks = []
            for sc in range(TC):
                m = cpool.tile([P, T], F32, name=f"mask{sc}")
                nc.gpsimd.memset(m, 1.0)
                nc.gpsimd.affine_select(
                    out=m,
                    in_=m,
                    compare_op=mybir.AluOpType.is_ge,
                    fill=0.0,
                    base=-(sc * P),
                    channel_multiplier=-1,
                    pattern=[[1, T]],
                )
                masks.append(m)

            # Load weights (natural [H, C] layout, 3 partition chunks each)
            wq_s, wk_s, wv_s = [], [], []
            for hc in range(CC):
                for lst, src, nm in (
                    (wq_s, wq_d, "wq"),
                    (wk_s, wk_d, "wk"),
                    (wv_s, wv_d, "wv"),
                ):
                    t_ = cpool.tile([P, C], F32, name=f"{nm}{hc}")
                    nc.sync.dma_start(t_, src[hc * P : (hc + 1) * P, :])
                    lst.append(t_)

            # G = (Wq.T @ Wk) * SCALE   tiles: [c1 partition chunk, c2 free]
            g_s = []
            for c1 in range(CC):
                pg = pmp.tile([P, 512], F32, name="pg", tag="pm")[:, :C]
                for hc in range(CC):
                    nc.tensor.matmul(
                        pg,
                        lhsT=wq_s[hc][:, c1 * P : (c1 + 1) * P],
                        rhs=wk_s[hc],
                        start=(hc == 0),
                        stop=(hc == CC - 1),
                    )
                g_t = cpool.tile([P, C], MM_DT, name=f"g{c1}")
                nc.vector.tensor_scalar_mul(g_t, pg, SCALE)
                g_s.append(g_t)

            # WvT tiles: [c partition chunk, h free]
            wvT_s = []
            for cc_ in range(CC):
                wvT = cpool.tile([P, H], MM_DT, name=f"wvT{cc_}")
                for hc in range(CC):
                    ptt = ptp.tile([P, P], F32, name="ptw", tag="pt")
                    nc.tensor.transpose(
                        ptt, wv_s[hc][:, cc_ * P : (cc_ + 1) * P], ident
                    )
                    nc.vector.tensor_copy(wvT[:, hc * P : (hc + 1) * P], ptt)
                wvT_s.append(wvT)

            for b in range(nb):
                # load x[b] -> 2 tiles [128, C]
                xa = []
                for tcc in range(TC):
                    xat = sb.tile([P, C], F32, name=f"xa{tcc}", tag=f"xa{tcc}")
                    nc.sync.dma_start(xat, x_d[b, tcc * P : (tcc + 1) * P, :])
                    xa.append(xat)

                # transpose x -> xT tiles [c chunk][128, T]
                xT = []
                for cc_ in range(CC):
                    xTt = sb.tile([P, T], MM_DT, name=f"xT{cc_}", tag=f"xT{cc_}")
                    xT.append(xTt)
                for tcc in range(TC):
                    for cc_ in range(CC):
                        ptt = ptp.tile([P, P], F32, name="ptx", tag="pt")
                        nc.tensor.transpose(
                            ptt, xa[tcc][:, cc_ * P : (cc_ + 1) * P], ident
                        )
                        nc.vector.tensor_copy(
                            xT[cc_][:, tcc * P : (tcc + 1) * P], ptt
                        )

                # z2[c2] = sum_c1 G[c1, c2-chunk] * xT[c1]   ([C, T], scaled)
                z2 = []
                for c2 in range(CC):
                    pz = pmp.tile([P, 512], F32, name="pz", tag="pm")[:, :T]
                    for c1 in range(CC):
                        nc.tensor.matmul(
                            pz,
                            lhsT=g_s[c1][:, c2 * P : (c2 + 1) * P],
                            rhs=xT[c1],
                            start=(c1 == 0),
                            stop=(c1 == CC - 1),
                        )
                    z2t = sb.tile([P, T], MM_DT, name=f"z2{c2}", tag=f"z2{c2}")
                    nc.vector.tensor_copy(z2t, pz)
                    z2.append(z2t)

                # v_aug[sc] = [x[b] @ Wv.T | 1]   ([128, H+1])
                vau = []
                for sc in range(TC):
                    pv = pmp.tile([P, 512], F32, name="pv", tag="pm")[:, :H]
                    for cc_ in range(CC):
                        nc.tensor.matmul(
                            pv,
                            lhsT=xT[cc_][:, sc * P : (sc + 1) * P],
                            rhs=wvT_s[cc_],
                            start=(cc_ == 0),
                            stop=(cc_ == CC - 1),
                        )
                    vt = sb.tile([P, H + 4], MM_DT, name=f"v{sc}", tag=f"v{sc}")
                    nc.vector.tensor_copy(vt[:, :H], pv)
                    # ones columns for the softmax-denominator trick (padded
                    # to 4 cols so the moving operand stays 16B-aligned),
                    # written via DVE so the values are f32r-rounded
                    nc.vector.tensor_scalar(
                        vt[:, H : H + 4],
                        masks[0][:, :4],
                        0.0,
                        1.0,
                        mybir.AluOpType.mult,
                        mybir.AluOpType.add,
                    )
                    vau.append(vt)

                # ST[s, t] = scaled scores transposed; exp + causal 0/1 mask
                est = []
                for sc in range(TC):
                    pst = pmp.tile([P, 512], F32, name="pst", tag="pm")[:, :T]
                    for cc_ in range(CC):
                        nc.tensor.matmul(
                            pst,
                            lhsT=xT[cc_][:, sc * P : (sc + 1) * P],
                            rhs=z2[cc_],
                            start=(cc_ == 0),
                            stop=(cc_ == CC - 1),
                        )
                    et = sb.tile([P, T], MM_DT, name=f"e{sc}", tag=f"e{sc}")
                    nc.scalar.activation(
                        et, pst, mybir.ActivationFunctionType.Exp
                    )
                    nc.vector.tensor_mul(et, et, masks[sc])
                    est.append(et)

                # out[t, h] = (sum_s est[s, t] * v_aug[s, h]) / r[t]
                for tcc in range(TC):
                    po = pmp.tile([P, 512], F32, name="po", tag="pm")[:, : H + 4]
                    for sc in range(TC):
                        nc.tensor.matmul(
                            po,
                            lhsT=est[sc][:, tcc * P : (tcc + 1) * P],
                            rhs=vau[sc],
                            start=(sc == 0),
                            stop=(sc == TC - 1),
                        )
                    rec = obp.tile([P, 1], F32, name="rec", tag="rec")
                    nc.vector.reciprocal(rec, po[:, H : H + 1])
                    ot = obp.tile([P, H], F32, name="ot", tag="ot")
                    nc.vector.tensor_scalar_mul(ot, po[:, :H], rec)
                    nc.sync.dma_start(out_d[b, tcc * P : (tcc + 1) * P, :], ot)

    nc.compile()
    return nc


_NC_CACHE = {}


def _get_nc(nb: int):
    if nb not in _NC_CACHE:
        _NC_CACHE[nb] = build_bass(nb)
    return _NC_CACHE[nb]


def kernel(x: np.ndarray, Wk: np.ndarray, Wq: np.ndarray, Wv: np.ndarray, **_):
    x = np.ascontiguousarray(x, dtype=np.float32)
    Wk = np.ascontiguousarray(Wk, dtype=np.float32)
    Wq = np.ascontiguousarray(Wq, dtype=np.float32)
    Wv = np.ascontiguousarray(Wv, dtype=np.float32)
    nb = x.shape[0] // NCORES
    nc = _get_nc(nb)
    in_maps = [
        {"x": x[i * nb : (i + 1) * nb], "Wk": Wk, "Wq": Wq, "Wv": Wv}
        for i in range(NCORES)
    ]
    res = run_bass_kernel_spmd(nc, in_maps, core_ids=list(range(NCORES)))
    return np.concatenate([r["out"] for r in res.results], axis=0)


if __name__ == "__main__":
    rng = np.random.default_rng(0)
    x = rng.standard_normal((B, T, C), dtype=np.float32)
    s = 1.0 / np.sqrt(C)
    Wk = rng.standard_normal((H, C), dtype=np.float32) * s
    Wq = rng.standard_normal((H, C), dtype=np.float32) * s
    Wv = rng.standard_normal((H, C), dtype=np.float32) * s
    out = kernel(x=x, Wk=Wk, Wq=Wq, Wv=Wv)
    print(out.shape, out.dtype)



# revision 2
# speedup vs baseline: 1.7899x; 1.7899x over previous
"""Single-head causal self-attention on 8 Trainium2 NeuronCores.

Reference computation (per batch b):
    k = x @ Wk.T ; q = x @ Wq.T ; v = x @ Wv.T
    wei = softmax(mask(q @ k.T / sqrt(H)))
    out = wei @ v

Strategy:
  - Data parallel: B=256 sharded across 8 cores (32 batches each), weights
    replicated. No cross-core communication.
  - Algebraic fusion: q @ k.T = x (Wq.T Wk) x.T. G = Wq.T @ Wk * scale is
    precomputed on the host, which halves the per-batch q/k projection work.
  - All per-batch operands are pre-laid-out on the host so the device never
    transposes: x is shipped as xT[b] = x[b].T (contraction dim on
    partitions), Wv as WvT = Wv.T. This removes 6 PE transposes + 6 vector
    copies per batch vs. transposing on-chip.
  - bf16 operands (f32 PSUM accumulation): 1 cycle/row matmuls at any free
    size, half the DMA bytes, and enough precision for the 2e-2 gate
    (measured ~2e-3).
  - Scores are computed in transposed layout ST[s, t] so exp(ST) feeds the
    output matmul as the stationary operand. Causal structure is exploited:
    the s-block [128:256) only produces t in [128:256) (N=128 matmuls), and
    the t-block [0:128) output sums over s in [0:128) only.
  - Softmax denominator: V is augmented with ones columns so the output
    matmul also yields r[t] = sum_s exp(ST[s,t]); normalization is a
    reciprocal + per-partition scale. No max-subtraction: logits are ~N(0,1).
  - Software-pipelined schedule (A: load+projections, B1: scores+exp+mask,
    B2: output+normalize+store) with a 2-batch skew so the ~0.6us exp/mask
    latency hides under the next batch's projection matmuls, and elementwise
    work is spread over ACT/DVE/GpSimd to keep PE the only near-saturated
    engine.
"""

import ml_dtypes
import numpy as np

import concourse.bass as bass
import concourse.mybir as mybir
from concourse import bacc
import concourse.tile as tile
from concourse.bass_utils import run_bass_kernel_spmd

B, T, C, H = 256, 256, 384, 384
NCORES = 8
NB = B // NCORES  # batches per core
P = 128
CC = C // P  # 3 chunks of the embedding dim
SCALE = float(H) ** -0.5
F32 = mybir.dt.float32
BF16 = mybir.dt.bfloat16
NPBF16 = ml_dtypes.bfloat16
VW = H + 8  # v augmented with 8 ones columns (16B-aligned in bf16)


def build_bass(nb: int = NB):
    nc = bacc.Bacc(
        "TRN2",
        target_bir_lowering=False,
        debug=False,
        enable_asserts=False,
        num_devices=NCORES,
    )
    xT_d = nc.dram_tensor("xT", [nb, C, T], BF16, kind="ExternalInput").ap()
    g_d = nc.dram_tensor("G", [C, C], BF16, kind="ExternalInput").ap()
    wvT_d = nc.dram_tensor("WvT", [C, H], BF16, kind="ExternalInput").ap()
    m_d = nc.dram_tensor("M", [P, P], BF16, kind="ExternalInput").ap()
    out_d = nc.dram_tensor("out", [nb, T, H], BF16, kind="ExternalOutput").ap()

    EXP = mybir.ActivationFunctionType.Exp
    CPY = mybir.ActivationFunctionType.Copy

    with tile.TileContext(nc) as tc:
        with (
            tc.tile_pool(name="const", bufs=1) as cpool,
            tc.tile_pool(name="io", bufs=4) as iop,
            tc.tile_pool(name="wk", bufs=4) as wkp,
            tc.tile_pool(name="ps", bufs=6, space="PSUM") as psp,
            tc.tile_pool(name="ps1", bufs=2, space="PSUM") as ps1p,
        ):
            g_s, wvT_s = [], []
            for i in range(CC):
                gt = cpool.tile([P, C], BF16, name=f"g{i}")
                nc.sync.dma_start(gt, g_d[i * P : (i + 1) * P, :])
                g_s.append(gt)
                wt = cpool.tile([P, H], BF16, name=f"wvT{i}")
                nc.sync.dma_start(wt, wvT_d[i * P : (i + 1) * P, :])
                wvT_s.append(wt)
            mtri = cpool.tile([P, P], BF16, name="mtri")
            nc.sync.dma_start(mtri, m_d)

            # per-batch state threaded between pipeline stages
            st_xT = [None] * nb
            st_z2 = [None] * nb
            st_vau = [None] * nb
            st_e0 = [None] * nb
            st_e1 = [None] * nb

            def stage_a(b):
                # loads + z2 = G @ x[b].T + v projections
                xT = []
                for cc_ in range(CC):
                    t_ = iop.tile([P, T], BF16, name=f"xT{cc_}", tag=f"xT{cc_}")
                    nc.sync.dma_start(t_, xT_d[b, cc_ * P : (cc_ + 1) * P, :])
                    xT.append(t_)
                st_xT[b] = xT

                z2 = []
                for c2 in range(CC):
                    pz = psp.tile([P, 512], F32, name="pz", tag="pm")[:, :T]
                    for c1 in range(CC):
                        nc.tensor.matmul(
                            pz,
                            lhsT=g_s[c1][:, c2 * P : (c2 + 1) * P],
                            rhs=xT[c1],
                            start=(c1 == 0),
                            stop=(c1 == CC - 1),
                        )
                    z2t = wkp.tile([P, T], BF16, name=f"z2{c2}", tag=f"z2{c2}")
                    nc.scalar.activation(z2t, pz, CPY)
                    z2.append(z2t)
                st_z2[b] = z2

                vau = []
                for sc in range(2):
                    pv = psp.tile([P, 512], F32, name="pv", tag="pm")[:, :H]
                    for cc_ in range(CC):
                        nc.tensor.matmul(
                            pv,
                            lhsT=xT[cc_][:, sc * P : (sc + 1) * P],
                            rhs=wvT_s[cc_],
                            start=(cc_ == 0),
                            stop=(cc_ == CC - 1),
                        )
                    vt = wkp.tile([P, VW], BF16, name=f"v{sc}", tag=f"v{sc}")
                    nc.gpsimd.memset(vt[:, H:VW], 1.0)
                    if sc == 0:
                        nc.scalar.activation(vt[:, :H], pv, CPY)
                    else:
                        nc.vector.tensor_copy(vt[:, :H], pv)
                    vau.append(vt)
                st_vau[b] = vau

            def stage_b1(b):
                # scores (transposed), exp, causal mask
                xT, z2 = st_xT[b], st_z2[b]
                pst0 = psp.tile([P, 512], F32, name="pst0", tag="pm")[:, :T]
                for cc_ in range(CC):
                    nc.tensor.matmul(
                        pst0,
                        lhsT=xT[cc_][:, 0:P],
                        rhs=z2[cc_],
                        start=(cc_ == 0),
                        stop=(cc_ == CC - 1),
                    )
                e0 = wkp.tile([P, T], BF16, name="e0", tag="e0")
                nc.scalar.activation(e0, pst0, EXP)
                nc.gpsimd.tensor_mul(e0[:, :P], e0[:, :P], mtri)
                st_e0[b] = e0

                pst1 = ps1p.tile([P, P], F32, name="pst1", tag="p1")
                for cc_ in range(CC):
                    nc.tensor.matmul(
                        pst1,
                        lhsT=xT[cc_][:, P : 2 * P],
                        rhs=z2[cc_][:, P : 2 * P],
                        start=(cc_ == 0),
                        stop=(cc_ == CC - 1),
                    )
                e1 = wkp.tile([P, P], BF16, name="e1", tag="e1")
                nc.scalar.activation(e1, pst1, EXP)
                nc.gpsimd.tensor_mul(e1, e1, mtri)
                st_e1[b] = e1

            def stage_b2(b):
                # output matmuls + softmax normalization + store
                e0, e1, vau = st_e0[b], st_e1[b], st_vau[b]
                st_xT[b] = st_z2[b] = st_vau[b] = st_e0[b] = st_e1[b] = None

                po0 = psp.tile([P, 512], F32, name="po0", tag="pm")[:, :VW]
                nc.tensor.matmul(po0, lhsT=e0[:, 0:P], rhs=vau[0], start=True, stop=True)
                rec0 = wkp.tile([P, 1], F32, name="rec0", tag="rec0")
                nc.vector.reciprocal(rec0, po0[:, H : H + 1])
                o0 = iop.tile([P, H], BF16, name="o0", tag="o0")
                nc.vector.tensor_scalar_mul(o0, po0[:, :H], rec0)
                nc.sync.dma_start(out_d[b, 0:P, :], o0)

                po1 = psp.tile([P, 512], F32, name="po1", tag="pm")[:, :VW]
                nc.tensor.matmul(po1, lhsT=e0[:, P : 2 * P], rhs=vau[0], start=True, stop=False)
                nc.tensor.matmul(po1, lhsT=e1, rhs=vau[1], start=False, stop=True)
                rec1 = wkp.tile([P, 1], F32, name="rec1", tag="rec1")
                nc.vector.reciprocal(rec1, po1[:, H : H + 1])
                o1 = iop.tile([P, H], BF16, name="o1", tag="o1")
                nc.vector.tensor_scalar_mul(o1, po1[:, :H], rec1)
                nc.sync.dma_start(out_d[b, P : 2 * P, :], o1)

            # 2-batch skew: exp/mask latency of batch b hides under batch
            # b+2's projection matmuls
            for k in range(nb + 3):
                if 0 <= k - 3 < nb:
                    stage_b2(k - 3)
                if 0 <= k - 2 < nb:
                    stage_b1(k - 2)
                if k < nb:
                    stage_a(k)

    nc.compile()
    return nc


_NC_CACHE = {}


def _get_nc(nb: int):
    if nb not in _NC_CACHE:
        _NC_CACHE[nb] = build_bass(nb)
    return _NC_CACHE[nb]


def kernel(x: np.ndarray, Wk: np.ndarray, Wq: np.ndarray, Wv: np.ndarray, **_):
    x = np.asarray(x, dtype=np.float32)
    Wk = np.asarray(Wk, dtype=np.float32)
    Wq = np.asarray(Wq, dtype=np.float32)
    Wv = np.asarray(Wv, dtype=np.float32)

    G = ((Wq.T @ Wk) * SCALE).astype(NPBF16)  # [C, C]
    WvT = np.ascontiguousarray(Wv.T).astype(NPBF16)  # [C, H]
    M = np.triu(np.ones((P, P), np.float32)).astype(NPBF16)
    xT = np.ascontiguousarray(x.transpose(0, 2, 1)).astype(NPBF16)  # [B, C, T]

    nb = x.shape[0] // NCORES
    nc = _get_nc(nb)
    in_maps = [
        {"xT": xT[i * nb : (i + 1) * nb], "G": G, "WvT": WvT, "M": M}
        for i in range(NCORES)
    ]
    res = run_bass_kernel_spmd(nc, in_maps, core_ids=list(range(NCORES)))
    out = np.concatenate([r["out"] for r in res.results], axis=0)
    return out.astype(np.float32)


if __name__ == "__main__":
    rng = np.random.default_rng(0)
    x = rng.standard_normal((B, T, C), dtype=np.float32)
    s = 1.0 / np.sqrt(C)
    Wk = rng.standard_normal((H, C), dtype=np.float32) * s
    Wq = rng.standard_normal((H, C), dtype=np.float32) * s
    Wv = rng.standard_normal((H, C), dtype=np.float32) * s
    out = kernel(x=x, Wk=Wk, Wq=Wq, Wv=Wv)
    print(out.shape, out.dtype)


# revision 4
# speedup vs baseline: 2.0985x; 1.1724x over previous
"""Single-head causal self-attention on 8 Trainium2 NeuronCores.

Reference computation (per batch b):
    k = x @ Wk.T ; q = x @ Wq.T ; v = x @ Wv.T
    wei = softmax(mask(q @ k.T / sqrt(H)))
    out = wei @ v

Strategy:
  - Data parallel: B=256 sharded across 8 cores (32 batches each), weights
    replicated. No cross-core communication.
  - Algebraic fusion: q @ k.T = x (Wq.T Wk) x.T. G = Wq.T @ Wk * scale is
    precomputed on the host, which halves the per-batch q/k projection work.
  - All per-batch operands are pre-laid-out on the host so the device never
    transposes: x is shipped as xT[b] = x[b].T (contraction dim on
    partitions), Wv as WvT = Wv.T.
  - bf16 operands (f32 PSUM accumulation): 1 cycle/row matmuls at any free
    size, half the DMA bytes, and enough precision for the 2e-2 gate
    (measured ~4.5e-3).
  - Batches are processed in PAIRS: one input DMA and one output DMA per
    pair (the sync engine pays a fixed ~650ns per DMA regardless of size),
    and the z2 = G @ x.T matmuls of the two batches fuse into N=512
    instructions (9 instead of 18 per pair) with N=512 PSUM->SBUF copies.
  - Scores are computed in transposed layout ST[s, t] so exp(ST) feeds the
    output matmul as the stationary operand. Causal structure is exploited:
    the s-block [128:256) only produces t in [128:256) (N=128 matmuls), and
    the t-block [0:128) output sums over s in [0:128) only.
  - Softmax denominator: V is augmented with ones columns so the output
    matmul also yields r[t] = sum_s exp(ST[s,t]); normalization is a
    reciprocal + per-partition scale. No max-subtraction: logits are ~N(0,1).
  - Software-pipelined schedule (A: load+projections, B1: scores+exp+mask,
    B2: output+normalize+store) with a 2-pair skew so exp/mask latency hides
    under the next pair's projection matmuls; elementwise work is spread
    over ACT/DVE/GpSimd so PE is the only near-saturated engine.
"""

import ml_dtypes
import numpy as np

import concourse.bass as bass
import concourse.mybir as mybir
from concourse import bacc
import concourse.tile as tile
from concourse.bass_utils import run_bass_kernel_spmd

B, T, C, H = 256, 256, 384, 384
NCORES = 8
NB = B // NCORES  # batches per core
P = 128
CC = C // P  # 3 chunks of the embedding dim
SCALE = float(H) ** -0.5
F32 = mybir.dt.float32
BF16 = mybir.dt.bfloat16
NPBF16 = ml_dtypes.bfloat16
VW = H + 8  # v augmented with 8 ones columns (16B-aligned in bf16)
XW = CC * 2 * T  # paired input tile width: col = j*512 + i*256 + t
OW = 2 * 2 * H  # paired output tile width: col = i*768 + tc*384 + h


def build_bass(nb: int = NB):
    assert nb % 2 == 0
    nb2 = nb // 2
    nc = bacc.Bacc(
        "TRN2",
        target_bir_lowering=False,
        debug=False,
        enable_asserts=False,
        num_devices=NCORES,
    )
    xt_d = nc.dram_tensor("xt", [nb2, P, XW], BF16, kind="ExternalInput").ap()
    g_d = nc.dram_tensor("G", [C, C], BF16, kind="ExternalInput").ap()
    wvT_d = nc.dram_tensor("WvT", [C, H], BF16, kind="ExternalInput").ap()
    m_d = nc.dram_tensor("M", [P, P], BF16, kind="ExternalInput").ap()
    out_d = nc.dram_tensor("out", [nb2, P, OW], BF16, kind="ExternalOutput").ap()

    EXP = mybir.ActivationFunctionType.Exp
    CPY = mybir.ActivationFunctionType.Copy

    with tile.TileContext(nc) as tc:
        with (
            tc.tile_pool(name="const", bufs=1) as cpool,
            tc.tile_pool(name="io", bufs=4) as iop,
            tc.tile_pool(name="wk", bufs=3) as wkp,
            tc.tile_pool(name="ps", bufs=6, space="PSUM") as psp,
            tc.tile_pool(name="ps1", bufs=2, space="PSUM") as ps1p,
        ):
            g_s, wvT_s = [], []
            for i in range(CC):
                gt = cpool.tile([P, C], BF16, name=f"g{i}")
                nc.sync.dma_start(gt, g_d[i * P : (i + 1) * P, :])
                g_s.append(gt)
                wt = cpool.tile([P, H], BF16, name=f"wvT{i}")
                nc.sync.dma_start(wt, wvT_d[i * P : (i + 1) * P, :])
                wvT_s.append(wt)
            mtri = cpool.tile([P, P], BF16, name="mtri")
            nc.sync.dma_start(mtri, m_d)

            # per-pair state threaded between pipeline stages
            st_xt = [None] * nb2
            st_z2 = [None] * nb2
            st_vau = [None] * nb2
            st_e0 = [None] * nb2
            st_e1 = [None] * nb2

            def xsl(xt, i, j, lo, hi):
                # lhsT slice of batch i, embed chunk j, seq cols [lo:hi)
                base = j * 2 * T + i * T
                return xt[:, base + lo : base + hi]

            def stage_a(pb):
                # paired load + fused z2 matmuls + per-batch v projections
                xt = iop.tile([P, XW], BF16, name="xt", tag="xt")
                nc.sync.dma_start(xt, xt_d[pb])
                st_xt[pb] = xt

                z2 = []
                for c2 in range(CC):
                    pz = psp.tile([P, 512], F32, name="pz", tag="pm")
                    for c1 in range(CC):
                        nc.tensor.matmul(
                            pz,
                            lhsT=g_s[c1][:, c2 * P : (c2 + 1) * P],
                            rhs=xt[:, c1 * 512 : (c1 + 1) * 512],
                            start=(c1 == 0),
                            stop=(c1 == CC - 1),
                        )
                    z2t = wkp.tile([P, 512], BF16, name=f"z2{c2}", tag=f"z2{c2}")
                    nc.scalar.activation(z2t, pz, CPY)
                    z2.append(z2t)
                st_z2[pb] = z2

                vau = []
                for i in range(2):
                    for sc in range(2):
                        pv = psp.tile([P, 512], F32, name="pv", tag="pm")[:, :H]
                        for j in range(CC):
                            nc.tensor.matmul(
                                pv,
                                lhsT=xsl(xt, i, j, sc * P, (sc + 1) * P),
                                rhs=wvT_s[j],
                                start=(j == 0),
                                stop=(j == CC - 1),
                            )
                        vt = wkp.tile([P, VW], BF16, name=f"v{i}{sc}", tag=f"v{i}{sc}")
                        nc.gpsimd.memset(vt[:, H:VW], 1.0)
                        if sc == 0:
                            nc.scalar.activation(vt[:, :H], pv, CPY)
                        else:
                            nc.vector.tensor_copy(vt[:, :H], pv)
                        vau.append(vt)
                st_vau[pb] = vau

            def stage_b1(pb):
                # scores (transposed), exp, causal mask for both batches
                xt, z2 = st_xt[pb], st_z2[pb]
                e0s, e1s = [], []
                for i in range(2):
                    pst0 = psp.tile([P, 512], F32, name="pst0", tag="pm")[:, :T]
                    for j in range(CC):
                        nc.tensor.matmul(
                            pst0,
                            lhsT=xsl(xt, i, j, 0, P),
                            rhs=z2[j][:, i * T : (i + 1) * T],
                            start=(j == 0),
                            stop=(j == CC - 1),
                        )
                    e0 = wkp.tile([P, T], BF16, name=f"e0{i}", tag=f"e0{i}")
                    nc.scalar.activation(e0, pst0, EXP)
                    nc.gpsimd.tensor_mul(e0[:, :P], e0[:, :P], mtri)
                    e0s.append(e0)

                    pst1 = ps1p.tile([P, P], F32, name="pst1", tag="p1")
                    for j in range(CC):
                        nc.tensor.matmul(
                            pst1,
                            lhsT=xsl(xt, i, j, P, 2 * P),
                            rhs=z2[j][:, i * T + P : (i + 1) * T],
                            start=(j == 0),
                            stop=(j == CC - 1),
                        )
                    e1 = wkp.tile([P, P], BF16, name=f"e1{i}", tag=f"e1{i}")
                    nc.scalar.activation(e1, pst1, EXP)
                    nc.gpsimd.tensor_mul(e1, e1, mtri)
                    e1s.append(e1)
                st_e0[pb], st_e1[pb] = e0s, e1s

            def stage_b2(pb):
                # output matmuls + softmax normalization + paired store
                e0s, e1s, vau = st_e0[pb], st_e1[pb], st_vau[pb]
                st_xt[pb] = st_z2[pb] = st_vau[pb] = st_e0[pb] = st_e1[pb] = None

                o = iop.tile([P, OW], BF16, name="o", tag="o")
                for i in range(2):
                    e0, e1 = e0s[i], e1s[i]
                    v0, v1 = vau[2 * i], vau[2 * i + 1]

                    po0 = psp.tile([P, 512], F32, name="po0", tag="pm")[:, :VW]
                    nc.tensor.matmul(po0, lhsT=e0[:, 0:P], rhs=v0, start=True, stop=True)
                    rec0 = wkp.tile([P, 1], F32, name=f"rec0{i}", tag=f"rec0{i}")
                    nc.vector.reciprocal(rec0, po0[:, H : H + 1])
                    nc.vector.tensor_scalar_mul(
                        o[:, i * 2 * H : i * 2 * H + H], po0[:, :H], rec0
                    )

                    po1 = psp.tile([P, 512], F32, name="po1", tag="pm")[:, :VW]
                    nc.tensor.matmul(po1, lhsT=e0[:, P : 2 * P], rhs=v0, start=True, stop=False)
                    nc.tensor.matmul(po1, lhsT=e1, rhs=v1, start=False, stop=True)
                    rec1 = wkp.tile([P, 1], F32, name=f"rec1{i}", tag=f"rec1{i}")
                    nc.vector.reciprocal(rec1, po1[:, H : H + 1])
                    nc.vector.tensor_scalar_mul(
                        o[:, i * 2 * H + H : (i + 1) * 2 * H], po1[:, :H], rec1
                    )
                nc.sync.dma_start(out_d[pb], o)

            # 2-pair skew: exp/mask latency of pair pb hides under pair
            # pb+2's projection matmuls
            for k in range(nb2 + 3):
                if 0 <= k - 3 < nb2:
                    stage_b2(k - 3)
                if 0 <= k - 2 < nb2:
                    stage_b1(k - 2)
                if k < nb2:
                    stage_a(k)

    nc.compile()
    return nc


_NC_CACHE = {}


def _get_nc(nb: int):
    if nb not in _NC_CACHE:
        _NC_CACHE[nb] = build_bass(nb)
    return _NC_CACHE[nb]


def _pack_inputs(x, Wk, Wq, Wv):
    G = ((Wq.T @ Wk) * SCALE).astype(NPBF16)  # [C, C]
    WvT = np.ascontiguousarray(Wv.T).astype(NPBF16)  # [C, H]
    M = np.triu(np.ones((P, P), np.float32)).astype(NPBF16)
    nB = x.shape[0]
    # xt[b2, p, j*512 + i*256 + t] = x[2*b2+i, t, j*128+p]
    xt = (
        x.transpose(0, 2, 1)
        .reshape(nB // 2, 2, CC, P, T)
        .transpose(0, 3, 2, 1, 4)
        .reshape(nB // 2, P, XW)
    )
    xt = np.ascontiguousarray(xt).astype(NPBF16)
    return xt, G, WvT, M


def _unpack_output(o, nB):
    # o[b2, p, i*768 + tc*384 + h] -> out[2*b2+i, tc*128+p, h]
    return (
        o.reshape(nB // 2, P, 2, 2, H)
        .transpose(0, 2, 3, 1, 4)
        .reshape(nB, T, H)
        .astype(np.float32)
    )


def kernel(x: np.ndarray, Wk: np.ndarray, Wq: np.ndarray, Wv: np.ndarray, **_):
    x = np.asarray(x, dtype=np.float32)
    Wk = np.asarray(Wk, dtype=np.float32)
    Wq = np.asarray(Wq, dtype=np.float32)
    Wv = np.asarray(Wv, dtype=np.float32)

    xt, G, WvT, M = _pack_inputs(x, Wk, Wq, Wv)
    nb = x.shape[0] // NCORES
    nc = _get_nc(nb)
    nb2 = nb // 2
    in_maps = [
        {"xt": xt[i * nb2 : (i + 1) * nb2], "G": G, "WvT": WvT, "M": M}
        for i in range(NCORES)
    ]
    res = run_bass_kernel_spmd(nc, in_maps, core_ids=list(range(NCORES)))
    o = np.concatenate([r["out"] for r in res.results], axis=0)
    return _unpack_output(o, x.shape[0])


if __name__ == "__main__":
    rng = np.random.default_rng(0)
    x = rng.standard_normal((B, T, C), dtype=np.float32)
    s = 1.0 / np.sqrt(C)
    Wk = rng.standard_normal((H, C), dtype=np.float32) * s
    Wq = rng.standard_normal((H, C), dtype=np.float32) * s
    Wv = rng.standard_normal((H, C), dtype=np.float32) * s
    out = kernel(x=x, Wk=Wk, Wq=Wq, Wv=Wv)
    print(out.shape, out.dtype)
